# revision 17
# baseline (speedup 1.0000x reference)
"""Trainium2 Bass kernel for nn_Aggregator (GNN message passing).

Computation (see reference):
  entity_agg = scatter_mean(entity_emb[tail] * weight[edge_type-1], head, N_ENT)
  score      = softmax(user_emb @ latent_emb.T, axis=1)
  user_agg   = segment_sum(vals * entity_emb[cols], rows, N_USERS)
  mix        = score @ (softmax(disen_weight_att) @ weight)
  user_agg   = user_agg * (1 + mix)
Returns (entity_agg[100000,128] f32, user_agg[50000,128] f32).

Strategy (8 NeuronCores, no collectives):
  - Sort edges by head, shard by head range (12500 heads/core); sort nnz by
    row, shard by user range (6250 users/core).  Each core owns a disjoint
    slice of both outputs.
  - Per core: stream 128-edge tiles.  Gather tail rows of a replicated fp16
    entity table from HBM by indirect DMA.  A tiny runtime probe checks
    whether the DGE consumes one offset per descriptor (multi-index batching);
    if so, gathers are issued per 16-tile group, otherwise one per tile (some
    runtimes consume only one row index per partition).
    weight[rel] rows are selected on-device: outer-product broadcast of rel
    ids -> one-hot(rel) -> matmul with the resident weight table.
    msg = gathered * weight[rel]; scatter = one-hot(local head) matmul
    accumulated in PSUM per 125-head output block, scaled by 1/deg on flush.
    The user/interaction stream is analogous with per-nnz val scaling and a
    (1+mix) flush, mix computed on-device per 125-user block.
  - SPMD uniformity: every (core, block) is padded to the same tile count so
    one NEFF serves all 8 cores; padded edges carry local-head -1 (one-hot
    row of zeros => no contribution).
"""

import math
import os
import sys

import numpy as np

for _p in ("/opt/trn_rl_repo",):
    if _p not in sys.path:
        sys.path.insert(0, _p)

from contextlib import ExitStack

import concourse.bass as bass
import concourse.tile as tile
from concourse import bacc, mybir
from concourse.bass import IndirectOffsetOnAxis
from concourse.bass_utils import run_bass_kernel_spmd
from concourse.masks import make_identity

dt = mybir.dt
F32 = dt.float32
F16 = dt.float16
I32 = dt.int32
ALU = mybir.AluOpType
ACTF = mybir.ActivationFunctionType

N_ENT, CH = 100000, 128
N_REL = 64
N_USERS = 50000
N_FACT = 4
NCORES = 8
BH = 125            # heads per entity output block (<=128 PSUM partitions)
NB_E = 100          # entity blocks per core  (100*125*8 = 100000)
BU = 125            # users per user output block
NB_U = 50           # user blocks per core    (50*125*8 = 50000)
RCH = 64            # tiles per rel-metadata chunk ([1, RCH*128] row loads)

_CACHE = {}
LAST_NC = None
LAST_RESULTS = None


def _build_nc(T_blk: int, T_ublk: int, rep: int = 1, batched: bool = False):
    """Build + compile the SPMD program (shared by all 8 cores).

    rep > 1 repeats the whole compute body (same I/O) for differential
    wall-clock timing of the on-device portion.
    """
    T_E = NB_E * T_blk
    T_U = NB_U * T_ublk
    CH_RE = math.ceil(T_E / RCH) * RCH * 128   # padded rel stream length
    CH_RU = math.ceil(T_U / RCH) * RCH * 128

    GSZ = 16
    nc = bacc.Bacc("TRN2", target_bir_lowering=False, debug=False)

    def din(name, shape, dtype):
        return nc.dram_tensor(name, shape, dtype, kind="ExternalInput").ap()

    ent16 = din("ent16", [N_ENT, CH], F16)
    tailsT = din("tailsT", [128, T_E], I32)
    lheadT = din("lheadT", [128, T_E], F32)
    relF = din("relF", [CH_RE], F32)
    invcnt = din("invcnt", [128, NB_E], F32)
    usr = din("usr", [NB_U * BU, CH], F32)
    colsT = din("colsT", [128, T_U], I32)
    luserT = din("luserT", [128, T_U], F32)
    valsT = din("valsT", [128, T_U], F32)
    latent = din("latent", [N_FACT, CH], F32)
    w32 = din("w32", [N_REL, CH], F32)
    datt = din("datt", [N_FACT, N_REL], F32)

    ent_out = nc.dram_tensor("ent_out", [NB_E * BH, CH], F32, kind="ExternalOutput").ap()
    usr_out = nc.dram_tensor("usr_out", [NB_U * BU, CH], F32, kind="ExternalOutput").ap()

    with tile.TileContext(nc) as tc, ExitStack() as ctx:
        per = ctx.enter_context(tc.tile_pool(name="per", bufs=1))
        sbg = ctx.enter_context(tc.tile_pool(name="sbg", bufs=3))
        sbs = ctx.enter_context(tc.tile_pool(name="sbs", bufs=8))
        sgg = ctx.enter_context(tc.tile_pool(name="sgg", bufs=4))
        pp = ctx.enter_context(tc.tile_pool(name="pp", bufs=2, space="PSUM"))
        pph = ctx.enter_context(tc.tile_pool(name="pph", bufs=1, space="PSUM"))

        # ---- persistent loads ----
        tails_sb = per.tile([128, T_E], I32, tag="tails")
        nc.sync.dma_start(tails_sb[:], tailsT[:])
        lhead_sb = per.tile([128, T_E], F32, tag="lhead")
        nc.sync.dma_start(lhead_sb[:], lheadT[:])
        invc_sb = per.tile([128, NB_E], F32, tag="invc")
        nc.sync.dma_start(invc_sb[:], invcnt[:])
        cols_sb = per.tile([128, T_U], I32, tag="cols")
        nc.sync.dma_start(cols_sb[:], colsT[:])
        luser_sb = per.tile([128, T_U], F32, tag="luser")
        nc.sync.dma_start(luser_sb[:], luserT[:])
        vals_sb = per.tile([128, T_U], F32, tag="vals")
        nc.sync.dma_start(vals_sb[:], valsT[:])
        w32_sb = per.tile([N_REL, CH], F32, tag="w32")
        nc.sync.dma_start(w32_sb[:], w32[:])
        datt_sb = per.tile([N_FACT, N_REL], F32, tag="datt")
        nc.sync.dma_start(datt_sb[:], datt[:])
        lat_sb = per.tile([N_FACT, CH], F32, tag="lat")
        nc.sync.dma_start(lat_sb[:], latent[:])

        ident = per.tile([128, 128], F32, tag="ident")
        make_identity(nc, ident[:])
        iota_i = per.tile([128, BH], I32, tag="iotai")
        nc.gpsimd.iota(iota_i[:], pattern=[[1, BH]], channel_multiplier=0)
        iota_f = per.tile([128, BH], F32, tag="iotaf")
        nc.vector.tensor_copy(iota_f[:], iota_i[:])
        iota_h = per.tile([128, BH], F16, tag="iotah")
        nc.vector.tensor_copy(iota_h[:], iota_i[:])
        # [64, 512] column-constant = partition index (for rel one-hot)
        riota_i = per.tile([N_REL, 512], I32, tag="riotai")
        nc.gpsimd.iota(riota_i[:], pattern=[[0, 512]], channel_multiplier=1)
        riota_f = per.tile([N_REL, 512], F32, tag="riotaf")
        nc.vector.tensor_copy(riota_f[:], riota_i[:])
        ones_sb = per.tile([1, N_REL], F32, tag="ones")
        nc.vector.memset(ones_sb[:], 1.0)
        ablate = os.environ.get("KERNEL_ABLATE", "")
        gdummy = per.tile([128, CH], F16, tag="gdummy")
        nc.vector.memset(gdummy[:], 0.5)
        sdummy = per.tile([128, BH], F16, tag="sdummy")
        nc.vector.memset(sdummy[:], 0.0)

        # latentT [CH, F]
        latT_ps = pph.tile([128, 128], F32, tag="ph")
        nc.tensor.transpose(latT_ps[:, :N_FACT], lat_sb[:], ident[:N_FACT, :N_FACT])
        latT_sb = per.tile([128, N_FACT], F32, tag="latT")
        nc.vector.tensor_copy(latT_sb[:], latT_ps[:, :N_FACT])

        # disen_weight = softmax(datt, axis=-1) @ weight   -> [F, CH]
        dmx = per.tile([N_FACT, 1], F32, tag="dmx")
        nc.vector.reduce_max(dmx[:], datt_sb[:], axis=mybir.AxisListType.X, negate=True)
        dexp = per.tile([N_FACT, N_REL], F32, tag="dexp")
        nc.scalar.activation(dexp[:], datt_sb[:], ACTF.Exp, bias=dmx[:, :1])
        dsm = per.tile([N_FACT, 1], F32, tag="dsm")
        nc.vector.reduce_sum(dsm[:], dexp[:], axis=mybir.AxisListType.X)
        drc = per.tile([N_FACT, 1], F32, tag="drc")
        nc.vector.reciprocal(drc[:], dsm[:])
        dsoft = per.tile([N_FACT, N_REL], F32, tag="dsoft")
        nc.vector.tensor_scalar(dsoft[:], dexp[:], drc[:, :1], None, op0=ALU.mult)
        dsoftT_ps = pph.tile([N_REL, 128], F32, tag="ph2")
        nc.tensor.transpose(dsoftT_ps[:, :N_FACT], dsoft[:], ident[:N_FACT, :N_FACT])
        dsoftT_sb = per.tile([N_REL, N_FACT], F32, tag="dsoftT")
        nc.vector.tensor_copy(dsoftT_sb[:], dsoftT_ps[:, :N_FACT])
        dw_ps = pph.tile([N_FACT, CH], F32, tag="ph")
        nc.tensor.matmul(dw_ps[:], lhsT=dsoftT_sb[:], rhs=w32_sb[:], start=True, stop=True)
        dw_sb = per.tile([N_FACT, CH], F32, tag="dw")
        nc.vector.tensor_copy(dw_sb[:], dw_ps[:])

        for _rep in range(rep):
            # ---- phase A: mix1 = 1 + score @ disen_weight ----
            mix1_sb = per.tile([128, NB_U * CH], F32, tag="mix1")
            for b in range(NB_U):
                u_sb = sbs.tile([BU, CH], F32, tag="ua")
                nc.sync.dma_start(u_sb[:], usr[b * BU:(b + 1) * BU, :])
                uT_ps = pph.tile([128, BU], F32, tag="ph")
                nc.tensor.transpose(uT_ps[:], u_sb[:], ident[:BU, :BU])
                uT_sb = sbs.tile([128, BU], F32, tag="ub")
                nc.vector.tensor_copy(uT_sb[:], uT_ps[:])
                sc_ps = pph.tile([BU, 128], F32, tag="ph2")
                nc.tensor.matmul(sc_ps[:, :N_FACT], lhsT=uT_sb[:], rhs=latT_sb[:],
                                 start=True, stop=True)
                smx = sbs.tile([BU, 1], F32, tag="uc")
                nc.vector.reduce_max(smx[:], sc_ps[:, :N_FACT],
                                     axis=mybir.AxisListType.X, negate=True)
                sexp = sbs.tile([BU, N_FACT], F32, tag="ud")
                nc.scalar.activation(sexp[:], sc_ps[:, :N_FACT], ACTF.Exp, bias=smx[:, :1])
                ssm = sbs.tile([BU, 1], F32, tag="ue")
                nc.vector.reduce_sum(ssm[:], sexp[:], axis=mybir.AxisListType.X)
                src = sbs.tile([BU, 1], F32, tag="uf")
                nc.vector.reciprocal(src[:], ssm[:])
                ssoft = sbs.tile([BU, N_FACT], F32, tag="ug")
                nc.vector.tensor_scalar(ssoft[:], sexp[:], src[:, :1], None, op0=ALU.mult)
                scT_ps = pph.tile([N_FACT, BU], F32, tag="ph")
                nc.tensor.transpose(scT_ps[:], ssoft[:], ident[:BU, :BU])
                scT_sb = sbs.tile([N_FACT, BU], F32, tag="uh")
                nc.vector.tensor_copy(scT_sb[:], scT_ps[:])
                mix_ps = pph.tile([BU, CH], F32, tag="ph2")
                nc.tensor.matmul(mix_ps[:], lhsT=scT_sb[:], rhs=dw_sb[:],
                                 start=True, stop=True)
                nc.vector.tensor_scalar(
                    mix1_sb[:BU, b * CH:(b + 1) * CH], mix_ps[:], 1.0, None, op0=ALU.add)

            # ---- KG entity stream ----
            g_grp = None
            relrow = None
            oneh = None
            wg_ps = None
            acc = None
            for tt in range(T_E):
                b, tloc = divmod(tt, T_blk)
                if tt % RCH == 0:
                    relrow = sbg.tile([1, RCH * 128], F32, tag="relrow")
                    nc.sync.dma_start(relrow[:], relF[None, tt * 128:(tt + RCH) * 128])
                r4 = tt % 4
                if r4 == 0 and "norel" not in ablate:
                    n_in = min(4, T_E - tt)
                    w = n_in * 128
                    co = (tt % RCH) * 128
                    relb_ps = pp.tile([N_REL, 512], F32, tag="rb")
                    nc.tensor.matmul(relb_ps[:, :w], lhsT=ones_sb[:],
                                     rhs=relrow[:, co:co + w], start=True, stop=True)
                    oneh = sbg.tile([N_REL, 512], F32, tag="oneh")
                    nc.vector.tensor_tensor(oneh[:, :w], riota_f[:, :w],
                                            relb_ps[:, :w], op=ALU.is_equal)
                    wg_ps = pp.tile([128, 512], F32, tag="wg")
                    for j in range(n_in):
                        nc.tensor.matmul(
                            wg_ps[:, j * 128:(j + 1) * 128],
                            lhsT=oneh[:, j * 128:(j + 1) * 128],
                            rhs=w32_sb[:], start=True, stop=True)
                    wg_sb = sbg.tile([128, 512], F16, tag="wgsb")
                    nc.scalar.activation(wg_sb[:, :w], wg_ps[:, :w], ACTF.Copy)
                if "nogather" in ablate:
                    g_t = gdummy
                else:
                    g_t = sbs.tile([128, CH], F16, tag="G")
                    nc.gpsimd.indirect_dma_start(
                        out=g_t[:], out_offset=None, in_=ent16[:],
                        in_offset=IndirectOffsetOnAxis(ap=tails_sb[:, tt:tt + 1], axis=0))
                if "norel" in ablate:
                    msg_t = sbs.tile([128, CH], F16, tag="msg")
                    nc.vector.tensor_copy(msg_t[:], g_t[:])
                else:
                    msg_t = sbs.tile([128, CH], F16, tag="msg")
                    nc.vector.tensor_tensor(msg_t[:], g_t[:],
                                            wg_sb[:, r4 * 128:(r4 + 1) * 128], op=ALU.mult)
                if tloc == 0:
                    acc = pp.tile([128, CH], F32, tag="acc")
                if "noS" in ablate:
                    s_t = sdummy
                else:
                    s_t = sbs.tile([128, BH], F16, tag="S")
                    nc.vector.tensor_scalar(
                        s_t[:], iota_h[:], lhead_sb[:, tt:tt + 1], None, op0=ALU.is_equal)
                nc.tensor.matmul(acc[:BH, :], lhsT=s_t[:], rhs=msg_t[:],
                                 start=(tloc == 0), stop=(tloc == T_blk - 1))
                if tloc == T_blk - 1:
                    o_sb = sbs.tile([BH, CH], F32, tag="osb")
                    nc.vector.tensor_scalar(
                        o_sb[:], acc[:BH, :], invc_sb[:BH, b:b + 1], None, op0=ALU.mult)
                    nc.sync.dma_start(ent_out[b * BH:(b + 1) * BH, :], o_sb[:])

            # ---- user/interaction stream ----
            for tt in range(T_U):
                b, tloc = divmod(tt, T_ublk)
                if "nogather" in ablate:
                    g_t = gdummy
                elif batched:
                    if tt % GSZ == 0:
                        ng = min(GSZ, T_U - tt)
                        g_grp = sgg.tile([128, GSZ * CH], F16, tag="Ggrp")
                        nc.gpsimd.indirect_dma_start(
                            out=g_grp[:, :ng * CH], out_offset=None, in_=ent16[:],
                            in_offset=IndirectOffsetOnAxis(
                                ap=cols_sb[:, tt:tt + ng], axis=0))
                    g_t = g_grp[:, (tt % GSZ) * CH:(tt % GSZ + 1) * CH]
                else:
                    g_t = sbs.tile([128, CH], F16, tag="G")
                    nc.gpsimd.indirect_dma_start(
                        out=g_t[:], out_offset=None, in_=ent16[:],
                        in_offset=IndirectOffsetOnAxis(ap=cols_sb[:, tt:tt + 1], axis=0))
                msgu = sbs.tile([128, CH], F16, tag="msg")
                nc.scalar.activation(msgu[:], g_t[:], ACTF.Copy,
                                     scale=vals_sb[:, tt:tt + 1])
                if tloc == 0:
                    acc = pp.tile([128, CH], F32, tag="acc")
                if "noS" in ablate:
                    s_t = sdummy
                else:
                    s_t = sbs.tile([128, BU], F16, tag="S")
                    nc.vector.tensor_scalar(
                        s_t[:], iota_h[:], luser_sb[:, tt:tt + 1], None, op0=ALU.is_equal)
                nc.tensor.matmul(acc[:BU, :], lhsT=s_t[:], rhs=msgu[:],
                                 start=(tloc == 0), stop=(tloc == T_ublk - 1))
                if tloc == T_ublk - 1:
                    o_sb = sbs.tile([BU, CH], F32, tag="osb")
                    nc.vector.tensor_tensor(
                        o_sb[:], acc[:BU, :], mix1_sb[:BU, b * CH:(b + 1) * CH],
                        op=ALU.mult)
                    nc.sync.dma_start(usr_out[b * BU:(b + 1) * BU, :], o_sb[:])

    nc.compile()
    return nc


def _prep_side(keys_sorted, payload_cols, n_blocks_total, blk_rows, T_blk):
    """Pack sorted per-edge payloads into [n_blocks_total, T_blk*128] slots."""
    n = keys_sorted.shape[0]
    blk = keys_sorted // blk_rows
    cnt = np.bincount(blk, minlength=n_blocks_total)
    starts = np.zeros(n_blocks_total + 1, np.int64)
    np.cumsum(cnt, out=starts[1:])
    pos = np.arange(n, dtype=np.int64) - starts[blk]
    dst = blk * (T_blk * 128) + pos
    out = {}
    for name, (arr, pad, dtp) in payload_cols.items():
        flat = np.full(n_blocks_total * T_blk * 128, pad, dtype=dtp)
        flat[dst] = arr
        out[name] = flat.reshape(n_blocks_total, T_blk * 128)
    return out


def _prep_inputs(entity_emb, user_emb, latent_emb, edge_index, edge_type,
                 weight, disen_weight_att, interact_rows, interact_cols,
                 interact_vals):
    entity_emb = np.ascontiguousarray(np.asarray(entity_emb, np.float32))
    user_emb = np.ascontiguousarray(np.asarray(user_emb, np.float32))
    latent_emb = np.ascontiguousarray(np.asarray(latent_emb, np.float32))
    weight = np.ascontiguousarray(np.asarray(weight, np.float32))
    disen_weight_att = np.ascontiguousarray(np.asarray(disen_weight_att, np.float32))
    interact_vals = np.asarray(interact_vals, np.float32)
    heads = np.asarray(edge_index[0], np.int64)
    tails = np.asarray(edge_index[1], np.int64)
    rels = np.asarray(edge_type, np.int64) - 1
    rows = np.asarray(interact_rows, np.int64)
    cols = np.asarray(interact_cols, np.int64)

    eord = np.argsort(heads, kind="stable")
    heads_s, tails_s, rels_s = heads[eord], tails[eord], rels[eord]
    nord = np.argsort(rows, kind="stable")
    rows_s, cols_s, vals_s = rows[nord], cols[nord], interact_vals[nord]

    nbe_tot = NCORES * NB_E
    nbu_tot = NCORES * NB_U
    cnt_e = np.bincount(heads_s // BH, minlength=nbe_tot)
    cnt_u = np.bincount(rows_s // BU, minlength=nbu_tot)
    T_blk = max(1, int(math.ceil(cnt_e.max() / 128)))
    T_ublk = max(1, int(math.ceil(cnt_u.max() / 128)))

    epack = _prep_side(heads_s, {
        "tails": (tails_s.astype(np.int32), 0, np.int32),
        "lhead": ((heads_s % BH).astype(np.float32), -1.0, np.float32),
        "rel": (rels_s.astype(np.float32), 0.0, np.float32),
    }, nbe_tot, BH, T_blk)
    upack = _prep_side(rows_s, {
        "cols": (cols_s.astype(np.int32), 0, np.int32),
        "luser": ((rows_s % BU).astype(np.float32), -1.0, np.float32),
        "vals": (vals_s.astype(np.float32), 0.0, np.float32),
    }, nbu_tot, BU, T_ublk)

    deg = np.bincount(heads, minlength=NCORES * NB_E * BH).astype(np.float32)
    invc_full = (1.0 / np.maximum(deg, 1.0)).astype(np.float32)

    T_E = NB_E * T_blk
    T_U = NB_U * T_ublk
    CH_RE = math.ceil(T_E / RCH) * RCH * 128
    CH_RU = math.ceil(T_U / RCH) * RCH * 128
    ent16_np = entity_emb.astype(np.float16)

    in_maps = []
    for c in range(NCORES):
        et = epack["tails"][c * NB_E:(c + 1) * NB_E].reshape(T_E, 128)
        el = epack["lhead"][c * NB_E:(c + 1) * NB_E].reshape(T_E, 128)
        er = epack["rel"][c * NB_E:(c + 1) * NB_E].reshape(T_E * 128)
        relF = np.zeros(CH_RE, np.float32)
        relF[:T_E * 128] = er
        ut = upack["cols"][c * NB_U:(c + 1) * NB_U].reshape(T_U, 128)
        ul = upack["luser"][c * NB_U:(c + 1) * NB_U].reshape(T_U, 128)
        uv = upack["vals"][c * NB_U:(c + 1) * NB_U].reshape(T_U, 128)
        invc_c = invc_full[c * NB_E * BH:(c + 1) * NB_E * BH].reshape(NB_E, BH)
        invc_t = np.zeros((128, NB_E), np.float32)
        invc_t[:BH] = invc_c.T
        in_maps.append({
            "ent16": ent16_np,
            "tailsT": np.ascontiguousarray(et.T),
            "lheadT": np.ascontiguousarray(el.T),
            "relF": relF,
            "invcnt": invc_t,
            "usr": user_emb[c * NB_U * BU:(c + 1) * NB_U * BU],
            "colsT": np.ascontiguousarray(ut.T),
            "luserT": np.ascontiguousarray(ul.T),
            "valsT": np.ascontiguousarray(uv.T),
            "latent": latent_emb,
            "w32": weight,
            "datt": disen_weight_att,
        })
    return in_maps, T_blk, T_ublk


_BATCH_OK = None


def _probe_batched_gather():
    """Does this runtime's indirect DMA consume one offset per descriptor
    (multi-index batching), or only one per partition?  Decides which kernel
    variant to build.  Cached per process."""
    global _BATCH_OK
    if _BATCH_OK is not None:
        return _BATCH_OK
    if os.environ.get("KERNEL_FORCE_MODE") == "batched":
        _BATCH_OK = True
        return True
    if os.environ.get("KERNEL_FORCE_MODE") == "pertile":
        _BATCH_OK = False
        return False
    try:
        nc = bacc.Bacc("TRN2", target_bir_lowering=False, debug=False)
        tbl = nc.dram_tensor("tbl", [512, 4], F32, kind="ExternalInput").ap()
        idxd = nc.dram_tensor("idxd", [128, 4], I32, kind="ExternalInput").ap()
        outd = nc.dram_tensor("outp", [128, 16], F32, kind="ExternalOutput").ap()
        with tile.TileContext(nc) as tc:
            with tc.tile_pool(name="sb", bufs=1) as sb:
                idx_sb = sb.tile([128, 4], I32, tag="i")
                nc.sync.dma_start(idx_sb[:], idxd[:])
                g = sb.tile([128, 16], F32, tag="g")
                nc.gpsimd.indirect_dma_start(
                    out=g[:], out_offset=None, in_=tbl[:],
                    in_offset=IndirectOffsetOnAxis(ap=idx_sb[:], axis=0))
                nc.sync.dma_start(outd[:], g[:])
        nc.compile()
        rng = np.random.default_rng(0)
        tblv = np.arange(512 * 4, dtype=np.float32).reshape(512, 4)
        idxv = rng.permutation(512)[:512].astype(np.int32)[:512]
        idxv = idxv[:512][: 128 * 4].reshape(128, 4)
        res = run_bass_kernel_spmd(nc, [{"tbl": tblv, "idxd": idxv}], core_ids=[0])
        got = res.results[0]["outp"].reshape(128, 4, 4)
        _BATCH_OK = bool(np.array_equal(got, tblv[idxv]))
    except Exception:
        _BATCH_OK = False
    return _BATCH_OK


def kernel(**inputs):
    global LAST_RESULTS, LAST_NC
    in_maps, T_blk, T_ublk = _prep_inputs(**inputs)
    rep = int(os.environ.get("KERNEL_REP", "1"))
    batched = _probe_batched_gather() if os.environ.get("KERNEL_SIM", "0") != "1" \
        else (os.environ.get("KERNEL_FORCE_MODE") == "batched")
    key = (T_blk, T_ublk, rep, batched)
    if key not in _CACHE:
        _CACHE[key] = _build_nc(T_blk, T_ublk, rep=rep, batched=batched)
    nc = _CACHE[key]
    LAST_NC = nc

    if os.environ.get("KERNEL_SIM", "0") == "1":
        from concourse.bass_interp import CoreSim
        outs = []
        for c in range(NCORES):
            sim = CoreSim(nc, trace=False)
            for name, arr in in_maps[c].items():
                sim.tensor(name)[:] = arr
            sim.simulate(check_with_hw=False)
            outs.append({
                "ent_out": np.array(sim.tensor("ent_out")),
                "usr_out": np.array(sim.tensor("usr_out")),
            })
        ent = np.concatenate([o["ent_out"] for o in outs], axis=0)
        usr = np.concatenate([o["usr_out"] for o in outs], axis=0)
        return ent, usr

    res = run_bass_kernel_spmd(nc, in_maps, core_ids=list(range(NCORES)))
    LAST_RESULTS = res
    ent = np.concatenate([res.results[c]["ent_out"] for c in range(NCORES)], axis=0)
    usr = np.concatenate([res.results[c]["usr_out"] for c in range(NCORES)], axis=0)
    return ent, usr


# revision 22
# speedup vs baseline: 1.0042x; 1.0042x over previous
"""Trainium2 Bass kernel for nn_Aggregator (GNN message passing).

Computation (see reference):
  entity_agg = scatter_mean(entity_emb[tail] * weight[edge_type-1], head, N_ENT)
  score      = softmax(user_emb @ latent_emb.T, axis=1)
  user_agg   = segment_sum(vals * entity_emb[cols], rows, N_USERS)
  mix        = score @ (softmax(disen_weight_att) @ weight)
  user_agg   = user_agg * (1 + mix)
Returns (entity_agg[100000,128] f32, user_agg[50000,128] f32).

Strategy (8 NeuronCores, no collectives):
  - Sort edges by head, shard by head range (12500 heads/core); sort nnz by
    row, shard by user range (6250 users/core).  Each core owns a disjoint
    slice of both outputs.
  - Per core: stream 128-edge tiles.  Gather tail rows of a replicated fp16
    entity table from HBM by indirect DMA.  A tiny runtime probe checks
    whether the DGE consumes one offset per descriptor (multi-index batching);
    if so, gathers are issued per 16-tile group, otherwise one per tile (some
    runtimes consume only one row index per partition).
    weight[rel] rows are selected on-device: outer-product broadcast of rel
    ids -> one-hot(rel) -> matmul with the resident weight table.
    msg = gathered * weight[rel]; scatter = one-hot(local head) matmul
    accumulated in PSUM per 125-head output block, scaled by 1/deg on flush.
    The user/interaction stream is analogous with per-nnz val scaling and a
    (1+mix) flush, mix computed on-device per 125-user block.
  - SPMD uniformity: every (core, block) is padded to the same tile count so
    one NEFF serves all 8 cores; padded edges carry local-head -1 (one-hot
    row of zeros => no contribution).
"""

import math
import os
import sys

import numpy as np

for _p in ("/opt/trn_rl_repo",):
    if _p not in sys.path:
        sys.path.insert(0, _p)

from contextlib import ExitStack

import concourse.bass as bass
import concourse.tile as tile
from concourse import bacc, mybir
from concourse.bass import IndirectOffsetOnAxis
from concourse.bass_utils import run_bass_kernel_spmd
from concourse.masks import make_identity

dt = mybir.dt
F32 = dt.float32
F16 = dt.float16
I32 = dt.int32
ALU = mybir.AluOpType
ACTF = mybir.ActivationFunctionType

N_ENT, CH = 100000, 128
N_REL = 64
N_USERS = 50000
N_FACT = 4
NCORES = 8
BH = 125            # heads per entity output block (<=128 PSUM partitions)
NB_E = 100          # entity blocks per core  (100*125*8 = 100000)
BU = 125            # users per user output block
NB_U = 50           # user blocks per core    (50*125*8 = 50000)
RCH = 64            # tiles per rel-metadata chunk ([1, RCH*128] row loads)

_CACHE = {}
LAST_NC = None
LAST_RESULTS = None


def _build_nc(T_blk: int, T_ublk: int, rep: int = 1, batched: bool = False):
    """Build + compile the SPMD program (shared by all 8 cores).

    rep > 1 repeats the whole compute body (same I/O) for differential
    wall-clock timing of the on-device portion.
    """
    T_E = NB_E * T_blk
    T_U = NB_U * T_ublk
    CH_RE = math.ceil(T_E / RCH) * RCH * 128   # padded rel stream length
    CH_RU = math.ceil(T_U / RCH) * RCH * 128

    GSZ = 16
    nc = bacc.Bacc("TRN2", target_bir_lowering=False, debug=False)

    def din(name, shape, dtype):
        return nc.dram_tensor(name, shape, dtype, kind="ExternalInput").ap()

    ent16 = din("ent16", [N_ENT, CH], F16)
    tailsT = din("tailsT", [128, T_E], I32)
    lheadT = din("lheadT", [128, T_E], F32)
    G32 = math.ceil(T_E / 32)
    ohrel = din("ohrel", [G32, N_REL, 32 * 128], F16)
    w16 = din("w16", [N_REL, CH], F16)
    invcnt = din("invcnt", [128, NB_E], F32)
    usr = din("usr", [NB_U * BU, CH], F32)
    colsT = din("colsT", [128, T_U], I32)
    luserT = din("luserT", [128, T_U], F32)
    valsT = din("valsT", [128, T_U], F32)
    latent = din("latent", [N_FACT, CH], F32)
    w32 = din("w32", [N_REL, CH], F32)
    datt = din("datt", [N_FACT, N_REL], F32)

    ent_out = nc.dram_tensor("ent_out", [NB_E * BH, CH], F32, kind="ExternalOutput").ap()
    usr_out = nc.dram_tensor("usr_out", [NB_U * BU, CH], F32, kind="ExternalOutput").ap()

    with tile.TileContext(nc) as tc, ExitStack() as ctx:
        per = ctx.enter_context(tc.tile_pool(name="per", bufs=1))
        sbg = ctx.enter_context(tc.tile_pool(name="sbg", bufs=3))
        sbs = ctx.enter_context(tc.tile_pool(name="sbs", bufs=8))
        sgg = ctx.enter_context(tc.tile_pool(name="sgg", bufs=4))
        pp = ctx.enter_context(tc.tile_pool(name="pp", bufs=2, space="PSUM"))
        pph = ctx.enter_context(tc.tile_pool(name="pph", bufs=1, space="PSUM"))

        # ---- persistent loads ----
        tails_sb = per.tile([128, T_E], I32, tag="tails")
        nc.sync.dma_start(tails_sb[:], tailsT[:])
        lhead_sb = per.tile([128, T_E], F32, tag="lhead")
        nc.sync.dma_start(lhead_sb[:], lheadT[:])
        invc_sb = per.tile([128, NB_E], F32, tag="invc")
        nc.sync.dma_start(invc_sb[:], invcnt[:])
        cols_sb = per.tile([128, T_U], I32, tag="cols")
        nc.sync.dma_start(cols_sb[:], colsT[:])
        luser_sb = per.tile([128, T_U], F32, tag="luser")
        nc.sync.dma_start(luser_sb[:], luserT[:])
        vals_sb = per.tile([128, T_U], F32, tag="vals")
        nc.sync.dma_start(vals_sb[:], valsT[:])
        w32_sb = per.tile([N_REL, CH], F32, tag="w32")
        nc.sync.dma_start(w32_sb[:], w32[:])
        w16_sb = per.tile([N_REL, CH], F16, tag="w16")
        nc.sync.dma_start(w16_sb[:], w16[:])
        datt_sb = per.tile([N_FACT, N_REL], F32, tag="datt")
        nc.sync.dma_start(datt_sb[:], datt[:])
        lat_sb = per.tile([N_FACT, CH], F32, tag="lat")
        nc.sync.dma_start(lat_sb[:], latent[:])

        ident = per.tile([128, 128], F32, tag="ident")
        make_identity(nc, ident[:])
        iota_i = per.tile([128, BH], I32, tag="iotai")
        nc.gpsimd.iota(iota_i[:], pattern=[[1, BH]], channel_multiplier=0)
        iota_f = per.tile([128, BH], F32, tag="iotaf")
        nc.vector.tensor_copy(iota_f[:], iota_i[:])
        iota_h = per.tile([128, BH], F16, tag="iotah")
        nc.vector.tensor_copy(iota_h[:], iota_i[:])
        ablate = os.environ.get("KERNEL_ABLATE", "")
        gdummy = per.tile([128, CH], F16, tag="gdummy")
        nc.vector.memset(gdummy[:], 0.5)
        sdummy = per.tile([128, BH], F16, tag="sdummy")
        nc.vector.memset(sdummy[:], 0.0)

        # latentT [CH, F]
        latT_ps = pph.tile([128, 128], F32, tag="ph")
        nc.tensor.transpose(latT_ps[:, :N_FACT], lat_sb[:], ident[:N_FACT, :N_FACT])
        latT_sb = per.tile([128, N_FACT], F32, tag="latT")
        nc.vector.tensor_copy(latT_sb[:], latT_ps[:, :N_FACT])

        # disen_weight = softmax(datt, axis=-1) @ weight   -> [F, CH]
        dmx = per.tile([N_FACT, 1], F32, tag="dmx")
        nc.vector.reduce_max(dmx[:], datt_sb[:], axis=mybir.AxisListType.X, negate=True)
        dexp = per.tile([N_FACT, N_REL], F32, tag="dexp")
        nc.scalar.activation(dexp[:], datt_sb[:], ACTF.Exp, bias=dmx[:, :1])
        dsm = per.tile([N_FACT, 1], F32, tag="dsm")
        nc.vector.reduce_sum(dsm[:], dexp[:], axis=mybir.AxisListType.X)
        drc = per.tile([N_FACT, 1], F32, tag="drc")
        nc.vector.reciprocal(drc[:], dsm[:])
        dsoft = per.tile([N_FACT, N_REL], F32, tag="dsoft")
        nc.vector.tensor_scalar(dsoft[:], dexp[:], drc[:, :1], None, op0=ALU.mult)
        dsoftT_ps = pph.tile([N_REL, 128], F32, tag="ph2")
        nc.tensor.transpose(dsoftT_ps[:, :N_FACT], dsoft[:], ident[:N_FACT, :N_FACT])
        dsoftT_sb = per.tile([N_REL, N_FACT], F32, tag="dsoftT")
        nc.vector.tensor_copy(dsoftT_sb[:], dsoftT_ps[:, :N_FACT])
        dw_ps = pph.tile([N_FACT, CH], F32, tag="ph")
        nc.tensor.matmul(dw_ps[:], lhsT=dsoftT_sb[:], rhs=w32_sb[:], start=True, stop=True)
        dw_sb = per.tile([N_FACT, CH], F32, tag="dw")
        nc.vector.tensor_copy(dw_sb[:], dw_ps[:])

        for _rep in range(rep):
            # ---- phase A: mix1 = 1 + score @ disen_weight ----
            mix1_sb = per.tile([128, NB_U * CH], F32, tag="mix1")
            for b in range(NB_U):
                u_sb = sbs.tile([BU, CH], F32, tag="ua")
                nc.sync.dma_start(u_sb[:], usr[b * BU:(b + 1) * BU, :])
                uT_ps = pph.tile([128, BU], F32, tag="ph")
                nc.tensor.transpose(uT_ps[:], u_sb[:], ident[:BU, :BU])
                uT_sb = sbs.tile([128, BU], F32, tag="ub")
                nc.vector.tensor_copy(uT_sb[:], uT_ps[:])
                sc_ps = pph.tile([BU, 128], F32, tag="ph2")
                nc.tensor.matmul(sc_ps[:, :N_FACT], lhsT=uT_sb[:], rhs=latT_sb[:],
                                 start=True, stop=True)
                smx = sbs.tile([BU, 1], F32, tag="uc")
                nc.vector.reduce_max(smx[:], sc_ps[:, :N_FACT],
                                     axis=mybir.AxisListType.X, negate=True)
                sexp = sbs.tile([BU, N_FACT], F32, tag="ud")
                nc.scalar.activation(sexp[:], sc_ps[:, :N_FACT], ACTF.Exp, bias=smx[:, :1])
                ssm = sbs.tile([BU, 1], F32, tag="ue")
                nc.vector.reduce_sum(ssm[:], sexp[:], axis=mybir.AxisListType.X)
                src = sbs.tile([BU, 1], F32, tag="uf")
                nc.vector.reciprocal(src[:], ssm[:])
                ssoft = sbs.tile([BU, N_FACT], F32, tag="ug")
                nc.vector.tensor_scalar(ssoft[:], sexp[:], src[:, :1], None, op0=ALU.mult)
                scT_ps = pph.tile([N_FACT, BU], F32, tag="ph")
                nc.tensor.transpose(scT_ps[:], ssoft[:], ident[:BU, :BU])
                scT_sb = sbs.tile([N_FACT, BU], F32, tag="uh")
                nc.vector.tensor_copy(scT_sb[:], scT_ps[:])
                mix_ps = pph.tile([BU, CH], F32, tag="ph2")
                nc.tensor.matmul(mix_ps[:], lhsT=scT_sb[:], rhs=dw_sb[:],
                                 start=True, stop=True)
                nc.vector.tensor_scalar(
                    mix1_sb[:BU, b * CH:(b + 1) * CH], mix_ps[:], 1.0, None, op0=ALU.add)

            # ---- KG entity stream ----
            g_grp = None
            oh_sb = None
            wg_ps = None
            acc = None
            for tt in range(T_E):
                b, tloc = divmod(tt, T_blk)
                if tt % 32 == 0 and "norel" not in ablate:
                    oh_sb = sbg.tile([N_REL, 32 * 128], F16, tag="ohrel")
                    nc.sync.dma_start(oh_sb[:], ohrel[tt // 32])
                r4 = tt % 4
                if r4 == 0 and "norel" not in ablate:
                    n_in = min(4, T_E - tt)
                    w = n_in * 128
                    co = (tt % 32) * 128
                    wg_ps = pp.tile([128, 512], F32, tag="wg")
                    for j in range(n_in):
                        nc.tensor.matmul(
                            wg_ps[:, j * 128:(j + 1) * 128],
                            lhsT=oh_sb[:, co + j * 128:co + (j + 1) * 128],
                            rhs=w16_sb[:], start=True, stop=True)
                    wg_sb = sbg.tile([128, 512], F16, tag="wgsb")
                    nc.scalar.activation(wg_sb[:, :w], wg_ps[:, :w], ACTF.Copy)
                if "nogather" in ablate:
                    g_t = gdummy
                elif batched:
                    if tt % GSZ == 0:
                        ng = min(GSZ, T_E - tt)
                        g_grp = sgg.tile([128, GSZ * CH], F16, tag="Ggrp")
                        nc.gpsimd.indirect_dma_start(
                            out=g_grp[:, :ng * CH], out_offset=None, in_=ent16[:],
                            in_offset=IndirectOffsetOnAxis(
                                ap=tails_sb[:, tt:tt + ng], axis=0))
                    g_t = g_grp[:, (tt % GSZ) * CH:(tt % GSZ + 1) * CH]
                else:
                    g_t = sbs.tile([128, CH], F16, tag="G")
                    nc.gpsimd.indirect_dma_start(
                        out=g_t[:], out_offset=None, in_=ent16[:],
                        in_offset=IndirectOffsetOnAxis(ap=tails_sb[:, tt:tt + 1], axis=0))
                if "norel" in ablate:
                    msg_t = sbs.tile([128, CH], F16, tag="msg")
                    nc.vector.tensor_copy(msg_t[:], g_t[:])
                elif batched:
                    if r4 == 0:
                        w = min(4, T_E - tt) * 128
                        msg_grp = sbs.tile([128, 512], F16, tag="msg")
                        go = (tt % GSZ) * CH
                        nc.vector.tensor_tensor(
                            msg_grp[:, :w], g_grp[:, go:go + w], wg_sb[:, :w],
                            op=ALU.mult)
                    msg_t = msg_grp[:, r4 * 128:(r4 + 1) * 128]
                else:
                    msg_t = sbs.tile([128, CH], F16, tag="msg")
                    nc.vector.tensor_tensor(msg_t[:], g_t[:],
                                            wg_sb[:, r4 * 128:(r4 + 1) * 128], op=ALU.mult)
                if tloc == 0:
                    acc = pp.tile([128, CH], F32, tag="acc")
                if "noS" in ablate:
                    s_t = sdummy
                else:
                    s_t = sbs.tile([128, BH], F16, tag="S")
                    nc.vector.tensor_scalar(
                        s_t[:], iota_h[:], lhead_sb[:, tt:tt + 1], None, op0=ALU.is_equal)
                nc.tensor.matmul(acc[:BH, :], lhsT=s_t[:], rhs=msg_t[:],
                                 start=(tloc == 0), stop=(tloc == T_blk - 1))
                if tloc == T_blk - 1:
                    o_sb = sbs.tile([BH, CH], F32, tag="osb")
                    nc.vector.tensor_scalar(
                        o_sb[:], acc[:BH, :], invc_sb[:BH, b:b + 1], None, op0=ALU.mult)
                    nc.sync.dma_start(ent_out[b * BH:(b + 1) * BH, :], o_sb[:])

            # ---- user/interaction stream ----
            for tt in range(T_U):
                b, tloc = divmod(tt, T_ublk)
                if "nogather" in ablate:
                    g_t = gdummy
                elif batched:
                    if tt % GSZ == 0:
                        ng = min(GSZ, T_U - tt)
                        g_grp = sgg.tile([128, GSZ * CH], F16, tag="Ggrp")
                        nc.gpsimd.indirect_dma_start(
                            out=g_grp[:, :ng * CH], out_offset=None, in_=ent16[:],
                            in_offset=IndirectOffsetOnAxis(
                                ap=cols_sb[:, tt:tt + ng], axis=0))
                    g_t = g_grp[:, (tt % GSZ) * CH:(tt % GSZ + 1) * CH]
                else:
                    g_t = sbs.tile([128, CH], F16, tag="G")
                    nc.gpsimd.indirect_dma_start(
                        out=g_t[:], out_offset=None, in_=ent16[:],
                        in_offset=IndirectOffsetOnAxis(ap=cols_sb[:, tt:tt + 1], axis=0))
                msgu = sbs.tile([128, CH], F16, tag="msg")
                nc.vector.tensor_scalar(msgu[:], g_t[:], vals_sb[:, tt:tt + 1],
                                        None, op0=ALU.mult)
                if tloc == 0:
                    acc = pp.tile([128, CH], F32, tag="acc")
                if "noS" in ablate:
                    s_t = sdummy
                else:
                    s_t = sbs.tile([128, BU], F16, tag="S")
                    nc.vector.tensor_scalar(
                        s_t[:], iota_h[:], luser_sb[:, tt:tt + 1], None, op0=ALU.is_equal)
                nc.tensor.matmul(acc[:BU, :], lhsT=s_t[:], rhs=msgu[:],
                                 start=(tloc == 0), stop=(tloc == T_ublk - 1))
                if tloc == T_ublk - 1:
                    o_sb = sbs.tile([BU, CH], F32, tag="osb")
                    nc.vector.tensor_tensor(
                        o_sb[:], acc[:BU, :], mix1_sb[:BU, b * CH:(b + 1) * CH],
                        op=ALU.mult)
                    nc.sync.dma_start(usr_out[b * BU:(b + 1) * BU, :], o_sb[:])

    nc.compile()
    return nc


def _prep_side(keys_sorted, payload_cols, n_blocks_total, blk_rows, T_blk):
    """Pack sorted per-edge payloads into [n_blocks_total, T_blk*128] slots."""
    n = keys_sorted.shape[0]
    blk = keys_sorted // blk_rows
    cnt = np.bincount(blk, minlength=n_blocks_total)
    starts = np.zeros(n_blocks_total + 1, np.int64)
    np.cumsum(cnt, out=starts[1:])
    pos = np.arange(n, dtype=np.int64) - starts[blk]
    dst = blk * (T_blk * 128) + pos
    out = {}
    for name, (arr, pad, dtp) in payload_cols.items():
        flat = np.full(n_blocks_total * T_blk * 128, pad, dtype=dtp)
        flat[dst] = arr
        out[name] = flat.reshape(n_blocks_total, T_blk * 128)
    return out


def _prep_inputs(entity_emb, user_emb, latent_emb, edge_index, edge_type,
                 weight, disen_weight_att, interact_rows, interact_cols,
                 interact_vals):
    entity_emb = np.ascontiguousarray(np.asarray(entity_emb, np.float32))
    user_emb = np.ascontiguousarray(np.asarray(user_emb, np.float32))
    latent_emb = np.ascontiguousarray(np.asarray(latent_emb, np.float32))
    weight = np.ascontiguousarray(np.asarray(weight, np.float32))
    disen_weight_att = np.ascontiguousarray(np.asarray(disen_weight_att, np.float32))
    interact_vals = np.asarray(interact_vals, np.float32)
    heads = np.asarray(edge_index[0], np.int64)
    tails = np.asarray(edge_index[1], np.int64)
    rels = np.asarray(edge_type, np.int64) - 1
    rows = np.asarray(interact_rows, np.int64)
    cols = np.asarray(interact_cols, np.int64)

    eord = np.argsort(heads, kind="stable")
    heads_s, tails_s, rels_s = heads[eord], tails[eord], rels[eord]
    nord = np.argsort(rows, kind="stable")
    rows_s, cols_s, vals_s = rows[nord], cols[nord], interact_vals[nord]

    nbe_tot = NCORES * NB_E
    nbu_tot = NCORES * NB_U
    cnt_e = np.bincount(heads_s // BH, minlength=nbe_tot)
    cnt_u = np.bincount(rows_s // BU, minlength=nbu_tot)
    T_blk = max(1, int(math.ceil(cnt_e.max() / 128)))
    T_ublk = max(1, int(math.ceil(cnt_u.max() / 128)))

    epack = _prep_side(heads_s, {
        "tails": (tails_s.astype(np.int32), 0, np.int32),
        "lhead": ((heads_s % BH).astype(np.float32), -1.0, np.float32),
        "rel": (rels_s.astype(np.int16), -1, np.int16),
    }, nbe_tot, BH, T_blk)
    upack = _prep_side(rows_s, {
        "cols": (cols_s.astype(np.int32), 0, np.int32),
        "luser": ((rows_s % BU).astype(np.float32), -1.0, np.float32),
        "vals": (vals_s.astype(np.float32), 0.0, np.float32),
    }, nbu_tot, BU, T_ublk)

    deg = np.bincount(heads, minlength=NCORES * NB_E * BH).astype(np.float32)
    invc_full = (1.0 / np.maximum(deg, 1.0)).astype(np.float32)

    T_E = NB_E * T_blk
    T_U = NB_U * T_ublk
    CH_RE = math.ceil(T_E / RCH) * RCH * 128
    CH_RU = math.ceil(T_U / RCH) * RCH * 128
    ent16_np = entity_emb.astype(np.float16)
    w16_np = weight.astype(np.float16)
    G32 = math.ceil(T_E / 32)
    T32 = G32 * 32
    rel_iota = np.arange(64, dtype=np.int16)

    in_maps = []
    for c in range(NCORES):
        et = epack["tails"][c * NB_E:(c + 1) * NB_E].reshape(T_E, 128)
        el = epack["lhead"][c * NB_E:(c + 1) * NB_E].reshape(T_E, 128)
        er = epack["rel"][c * NB_E:(c + 1) * NB_E].reshape(T_E, 128)
        er_pad = np.full((T32, 128), -1, np.int16)
        er_pad[:T_E] = er
        oh = (er_pad[:, None, :] == rel_iota[None, :, None]).astype(np.float16)
        oh = (oh.reshape(G32, 32, 64, 128).transpose(0, 2, 1, 3)
                .reshape(G32, 64, 32 * 128))
        ut = upack["cols"][c * NB_U:(c + 1) * NB_U].reshape(T_U, 128)
        ul = upack["luser"][c * NB_U:(c + 1) * NB_U].reshape(T_U, 128)
        uv = upack["vals"][c * NB_U:(c + 1) * NB_U].reshape(T_U, 128)
        invc_c = invc_full[c * NB_E * BH:(c + 1) * NB_E * BH].reshape(NB_E, BH)
        invc_t = np.zeros((128, NB_E), np.float32)
        invc_t[:BH] = invc_c.T
        in_maps.append({
            "ent16": ent16_np,
            "tailsT": np.ascontiguousarray(et.T),
            "lheadT": np.ascontiguousarray(el.T),
            "ohrel": oh,
            "w16": w16_np,
            "invcnt": invc_t,
            "usr": user_emb[c * NB_U * BU:(c + 1) * NB_U * BU],
            "colsT": np.ascontiguousarray(ut.T),
            "luserT": np.ascontiguousarray(ul.T),
            "valsT": np.ascontiguousarray(uv.T),
            "latent": latent_emb,
            "w32": weight,
            "datt": disen_weight_att,
        })
    return in_maps, T_blk, T_ublk


_BATCH_OK = None


def _probe_batched_gather():
    """Does this runtime's indirect DMA consume one offset per descriptor
    (multi-index batching), or only one per partition?  Decides which kernel
    variant to build.  Cached per process."""
    global _BATCH_OK
    if _BATCH_OK is not None:
        return _BATCH_OK
    if os.environ.get("KERNEL_FORCE_MODE") == "batched":
        _BATCH_OK = True
        return True
    if os.environ.get("KERNEL_FORCE_MODE") == "pertile":
        _BATCH_OK = False
        return False
    try:
        nc = bacc.Bacc("TRN2", target_bir_lowering=False, debug=False)
        tbl = nc.dram_tensor("tbl", [512, 4], F32, kind="ExternalInput").ap()
        idxd = nc.dram_tensor("idxd", [128, 4], I32, kind="ExternalInput").ap()
        outd = nc.dram_tensor("outp", [128, 16], F32, kind="ExternalOutput").ap()
        with tile.TileContext(nc) as tc:
            with tc.tile_pool(name="sb", bufs=1) as sb:
                idx_sb = sb.tile([128, 4], I32, tag="i")
                nc.sync.dma_start(idx_sb[:], idxd[:])
                g = sb.tile([128, 16], F32, tag="g")
                nc.gpsimd.indirect_dma_start(
                    out=g[:], out_offset=None, in_=tbl[:],
                    in_offset=IndirectOffsetOnAxis(ap=idx_sb[:], axis=0))
                nc.sync.dma_start(outd[:], g[:])
        nc.compile()
        rng = np.random.default_rng(0)
        tblv = np.arange(512 * 4, dtype=np.float32).reshape(512, 4)
        idxv = rng.permutation(512)[:512].astype(np.int32)[:512]
        idxv = idxv[:512][: 128 * 4].reshape(128, 4)
        res = run_bass_kernel_spmd(nc, [{"tbl": tblv, "idxd": idxv}], core_ids=[0])
        got = res.results[0]["outp"].reshape(128, 4, 4)
        _BATCH_OK = bool(np.array_equal(got, tblv[idxv]))
    except Exception:
        _BATCH_OK = False
    return _BATCH_OK


def kernel(**inputs):
    global LAST_RESULTS, LAST_NC
    in_maps, T_blk, T_ublk = _prep_inputs(**inputs)
    rep = int(os.environ.get("KERNEL_REP", "1"))
    batched = _probe_batched_gather() if os.environ.get("KERNEL_SIM", "0") != "1" \
        else (os.environ.get("KERNEL_FORCE_MODE") == "batched")
    key = (T_blk, T_ublk, rep, batched)
    if key not in _CACHE:
        _CACHE[key] = _build_nc(T_blk, T_ublk, rep=rep, batched=batched)
    nc = _CACHE[key]
    LAST_NC = nc

    if os.environ.get("KERNEL_SIM", "0") == "1":
        from concourse.bass_interp import CoreSim
        outs = []
        for c in range(NCORES):
            sim = CoreSim(nc, trace=False)
            for name, arr in in_maps[c].items():
                sim.tensor(name)[:] = arr
            sim.simulate(check_with_hw=False)
            outs.append({
                "ent_out": np.array(sim.tensor("ent_out")),
                "usr_out": np.array(sim.tensor("usr_out")),
            })
        ent = np.concatenate([o["ent_out"] for o in outs], axis=0)
        usr = np.concatenate([o["usr_out"] for o in outs], axis=0)
        return ent, usr

    res = run_bass_kernel_spmd(nc, in_maps, core_ids=list(range(NCORES)))
    LAST_RESULTS = res
    ent = np.concatenate([res.results[c]["ent_out"] for c in range(NCORES)], axis=0)
    usr = np.concatenate([res.results[c]["usr_out"] for c in range(NCORES)], axis=0)
    return ent, usr
